# revision 3
# baseline (speedup 1.0000x reference)
"""BiAttention Trainium2 kernel (8 NeuronCores, data-parallel over batch).

Per core: 2 batches. Per batch b:
  C_ = lrelu(C @ W1.T + b1); Q_ = lrelu(Q @ W1.T + b1)     [fp32r matmuls]
  S  = C_ @ Q_.T, masked (Cmask rows / Qmask cols -> -1e30), softmax over Lq
  A  = S_ @ Q                                              [fp32r]
  cat = [C, A, A-C, A*C]
  fuse = tanh(cat @ Wf.T + bf); gate = sigmoid(cat @ Wg.T + bg)   [bf16]
  out = gate*fuse + (1-gate)*cat, rows with Cmask==0 -> -1e30

The 4-block cat GEMM is folded into 3 blocks on the host:
  cat @ W.T = C @ (W0-W2).T + A @ (W1+W2).T + (A*C) @ W3.T
where Wk = W[:, k*512:(k+1)*512].

Layouts (SBUF, partition dim first):
  ct_grp  [128, 4k x 512c]  fp32r  C^T for a 4-c-tile group (k = C feature chunk)
  cgrp    [128, 4d x 512c]  fp32r  C_^T for the group (d = output feature chunk)
  q_t     [128, 4d x 512q]  fp32r  Q_^T
  qb      [128, 4qc x 512d] fp32r  Q natural (q-chunk on partitions)
  wft/wgt [128, 12j x 2048f] bf16  folded weights, moving operand
"""
import sys

sys.path.insert(0, "/opt/trn_rl_repo")

import numpy as np
import ml_dtypes
from contextlib import ExitStack

import concourse.bass as bass
import concourse.tile as tile
from concourse import bacc, mybir
from concourse.bass_utils import run_bass_kernel_spmd
from concourse.masks import make_identity

F32 = mybir.dt.float32
F32R = mybir.dt.float32r
BF16 = mybir.dt.bfloat16
AF = mybir.ActivationFunctionType
ALU = mybir.AluOpType
AX = mybir.AxisListType
BF16_NP = ml_dtypes.bfloat16

N_CORES = 8
B_FULL, LC_FULL, LQ, D = 16, 2048, 512, 512
BPC = B_FULL // N_CORES


def _f32(ap):
    return ap.bitcast(F32)


def _body(ctx: ExitStack, tc, io, bpc, lc, gemm_bias):
    nc = tc.nc
    nct = lc // 128
    ngrp = nct // 4

    cons = ctx.enter_context(tc.tile_pool(name="cons", bufs=1))
    batch = ctx.enter_context(tc.tile_pool(name="batch", bufs=1))
    grp = ctx.enter_context(tc.tile_pool(name="grp", bufs=1))
    chain = ctx.enter_context(tc.tile_pool(name="chain", bufs=1))
    psA = ctx.enter_context(tc.tile_pool(name="psA", bufs=2, space="PSUM"))
    psB = ctx.enter_context(tc.tile_pool(name="psB", bufs=2, space="PSUM"))
    psG = ctx.enter_context(tc.tile_pool(name="psG", bufs=4, space="PSUM"))

    # ---- constants ----
    wft = cons.tile([128, 12 * 2048], BF16, tag="wft")
    wgt = cons.tile([128, 12 * 2048], BF16, tag="wgt")
    nc.sync.dma_start(wft[:], io["wft"])
    nc.sync.dma_start(wgt[:], io["wgt"])
    w1t = cons.tile([128, 4 * D], F32R, tag="w1t")
    with tc.tile_pool(name="init", bufs=1) as initp:
        w1t_f = initp.tile([128, 4 * D], F32, tag="w1t_f")
        nc.sync.dma_start(w1t_f[:], io["w1t"])
        nc.vector.tensor_copy(w1t[:], w1t_f[:])
    work = ctx.enter_context(tc.tile_pool(name="work", bufs=2))
    b1c = cons.tile([128, 4], F32, tag="b1c")
    nc.sync.dma_start(b1c[:], io["b1c"])
    ident = cons.tile([128, 128], F32, tag="ident")
    make_identity(nc, ident[:])
    ident_r = cons.tile([128, 128], F32R, tag="ident_r")
    nc.vector.tensor_copy(ident_r[:], ident[:])
    if gemm_bias:
        bfr = cons.tile([1, 2048], BF16, tag="bfr")
        bgr = cons.tile([1, 2048], BF16, tag="bgr")
        nc.sync.dma_start(bfr[:], io["bfr"])
        nc.sync.dma_start(bgr[:], io["bgr"])
        ones = cons.tile([1, 128], BF16, tag="ones")
        nc.vector.memset(ones[:], 1.0)

    for b in range(bpc):
        # ---- per-batch masks ----
        qma = batch.tile([128, LQ], F32, tag="qma")
        nc.sync.dma_start(qma[:], io["qma"][b])
        cmc = batch.tile([128, nct], F32, tag="cmc")
        nc.sync.dma_start(cmc[:], io["cmc"][b])
        bcc = batch.tile([128, nct], F32, tag="bcc")
        nc.sync.dma_start(bcc[:], io["bcc"][b])

        # ---- Q prep: qb (natural, fp32r), qt (Q^T), q_t (Q_^T) ----
        qb = batch.tile([128, 4 * D], F32R, tag="qb")
        qt = batch.tile([128, 4 * LQ], F32R, tag="qt")
        for qc in range(4):
            q_nat = work.tile([128, D], F32, tag="q_nat")
            nc.sync.dma_start(q_nat[:], io["q_in"][b, qc * 128:(qc + 1) * 128, :])
            nc.vector.tensor_copy(qb[:, qc * D:(qc + 1) * D], q_nat[:])
            ps = psB.tile([128, 512], F32, tag="psB")
            for k in range(4):
                nc.tensor.transpose(ps[:, k * 128:(k + 1) * 128],
                                    q_nat[:, k * 128:(k + 1) * 128], ident[:])
            # psum chunk k -> qt[:, k*512 + qc*128]
            dst = qt[:].rearrange("p (k q) -> p k q", k=4)[:, :,
                                                          qc * 128:(qc + 1) * 128]
            nc.vector.tensor_copy(dst, ps[:].rearrange("p (k q) -> p k q", k=4))
        q_t = batch.tile([128, 4 * LQ], F32R, tag="q_t")
        for dc in range(4):
            ps = psA.tile([128, 512], F32, tag="psA")
            for k in range(4):
                nc.tensor.matmul(ps[:], w1t[:, k * D + dc * 128:k * D + dc * 128 + 128],
                                 qt[:, k * LQ:(k + 1) * LQ],
                                 start=(k == 0), stop=(k == 3))
            tmp = chain.tile([128, 512], F32, tag="tmp")
            nc.scalar.activation(tmp[:], ps[:], AF.Identity,
                                 bias=b1c[:, dc:dc + 1], scale=1.0)
            nc.vector.scalar_tensor_tensor(q_t[:, dc * LQ:(dc + 1) * LQ],
                                           tmp[:], 0.01, tmp[:],
                                           op0=ALU.mult, op1=ALU.max)

        for g in range(ngrp):
            # ---- group prologue: C^T and C_^T for 4 c-tiles ----
            ct_grp = grp.tile([128, 4 * 512], F32R, tag="ct_grp")
            for t in range(4):
                ci = g * 4 + t
                c_nat = work.tile([128, D], F32, tag="c_nat0")
                nc.sync.dma_start(c_nat[:], io["c_in"][b, ci * 128:(ci + 1) * 128, :])
                ps = psB.tile([128, 512], F32, tag="psB")
                for k in range(4):
                    nc.tensor.transpose(ps[:, k * 128:(k + 1) * 128],
                                        c_nat[:, k * 128:(k + 1) * 128], ident[:])
                dst = ct_grp[:].rearrange("p (k c) -> p k c", k=4)[:, :,
                                                                  t * 128:(t + 1) * 128]
                nc.vector.tensor_copy(dst, ps[:].rearrange("p (k c) -> p k c", k=4))
            cgrp = grp.tile([128, 4 * 512], F32R, tag="cgrp")
            for dc in range(4):
                ps = psA.tile([128, 512], F32, tag="psA")
                for k in range(4):
                    nc.tensor.matmul(
                        ps[:], w1t[:, k * D + dc * 128:k * D + dc * 128 + 128],
                        ct_grp[:, k * 512:(k + 1) * 512],
                        start=(k == 0), stop=(k == 3))
                tmp = chain.tile([128, 512], F32, tag="tmp")
                nc.scalar.activation(tmp[:], ps[:], AF.Identity,
                                     bias=b1c[:, dc:dc + 1], scale=1.0)
                nc.vector.scalar_tensor_tensor(cgrp[:, dc * 512:(dc + 1) * 512],
                                               tmp[:], 0.01, tmp[:],
                                               op0=ALU.mult, op1=ALU.max)

            for t in range(4):
                ci = g * 4 + t
                # ---- S + masked softmax ----
                ps_s = psA.tile([128, 512], F32, tag="psA")
                for dc in range(4):
                    nc.tensor.matmul(
                        ps_s[:], cgrp[:, dc * 512 + t * 128:dc * 512 + t * 128 + 128],
                        q_t[:, dc * LQ:(dc + 1) * LQ],
                        start=(dc == 0), stop=(dc == 3))
                s1 = chain.tile([128, 512], F32, tag="s1")
                nc.vector.tensor_add(s1[:], ps_s[:], qma[:])
                s2 = chain.tile([128, 512], F32, tag="s2")
                nc.scalar.activation(s2[:], s1[:], AF.Identity,
                                     bias=bcc[:, ci:ci + 1], scale=cmc[:, ci:ci + 1])
                negm = chain.tile([128, 1], F32, tag="negm")
                nc.vector.reduce_max(negm[:], s2[:], axis=AX.X, negate=True)
                p_f = chain.tile([128, 512], F32, tag="p_f")
                ssum = chain.tile([128, 1], F32, tag="ssum")
                nc.scalar.activation(p_f[:], s2[:], AF.Exp, bias=negm[:], scale=1.0,
                                     accum_out=ssum[:])
                rec = chain.tile([128, 1], F32, tag="rec")
                nc.vector.reciprocal(rec[:], ssum[:])
                pb = chain.tile([128, 512], F32R, tag="pb")
                nc.scalar.activation(pb[:], p_f[:], AF.Copy, bias=0.0, scale=rec[:])
                # ---- P^T ----
                ps_pt = psB.tile([128, 512], F32R, tag="psB")
                for qc in range(4):
                    nc.tensor.transpose(ps_pt[:, qc * 128:(qc + 1) * 128],
                                        pb[:, qc * 128:(qc + 1) * 128], ident_r[:])
                pt = chain.tile([128, 512], F32R, tag="pt")
                nc.vector.tensor_copy(pt[:], _f32(ps_pt[:]))
                # ---- attn natural [c, d], then attn^T chunks ----
                ps_an = psB.tile([128, 512], F32, tag="psB")
                for qc in range(4):
                    nc.tensor.matmul(ps_an[:], pt[:, qc * 128:(qc + 1) * 128],
                                     qb[:, qc * D:(qc + 1) * D],
                                     start=(qc == 0), stop=(qc == 3))
                attn = work.tile([128, 512], F32R, tag="attn")
                nc.vector.tensor_copy(attn[:], ps_an[:])
                ps_at = psB.tile([128, 512], F32R, tag="psB")
                for dc in range(4):
                    nc.tensor.transpose(ps_at[:, dc * 128:(dc + 1) * 128],
                                        attn[:, dc * 128:(dc + 1) * 128], ident_r[:])
                attnt = work.tile([128, 512], BF16, tag="attnt")
                nc.vector.tensor_copy(attnt[:], _f32(ps_at[:]))
                ct_sl = _f32(ct_grp[:]).rearrange("p (k c) -> p k c", k=4)[
                    :, :, t * 128:(t + 1) * 128]
                multt = work.tile([128, 512], BF16, tag="multt")
                nc.vector.tensor_mul(multt[:].rearrange("p (k c) -> p k c", k=4),
                                     _f32(ps_at[:]).rearrange("p (k c) -> p k c", k=4),
                                     ct_sl)
                ctbf = work.tile([128, 512], BF16, tag="ctbf")
                nc.vector.tensor_copy(ctbf[:].rearrange("p (k c) -> p k c", k=4),
                                      ct_sl)
                # ---- cat natural blocks ----
                c_nat = work.tile([128, D], F32, tag="c_nat1")
                nc.sync.dma_start(c_nat[:], io["c_in"][b, ci * 128:(ci + 1) * 128, :])
                amc = work.tile([128, 512], F32, tag="amc")
                nc.vector.tensor_sub(amc[:], _f32(attn[:]), c_nat[:])
                amm = work.tile([128, 512], F32, tag="amm")
                nc.vector.tensor_mul(amm[:], _f32(attn[:]), c_nat[:])
                cat_blocks = [c_nat, None, amc, amm]
                # ---- the two big GEMMs + blend, per 512-wide f chunk ----
                for fc in range(4):
                    ps_f = psG.tile([128, 512], F32, tag="psG")
                    ps_g = psG.tile([128, 512], F32, tag="psG")
                    if gemm_bias:
                        nc.tensor.matmul(ps_f[:], ones[:, :],
                                         bfr[:, fc * 512:(fc + 1) * 512],
                                         start=True, stop=False)
                        nc.tensor.matmul(ps_g[:], ones[:, :],
                                         bgr[:, fc * 512:(fc + 1) * 512],
                                         start=True, stop=False)
                    for jc in range(12):
                        if jc < 4:
                            src = ctbf[:, (jc % 4) * 128:(jc % 4) * 128 + 128]
                        elif jc < 8:
                            src = attnt[:, (jc % 4) * 128:(jc % 4) * 128 + 128]
                        else:
                            src = multt[:, (jc % 4) * 128:(jc % 4) * 128 + 128]
                        st = (jc == 0) and not gemm_bias
                        nc.tensor.matmul(ps_f[:], src,
                                         wft[:, jc * 2048 + fc * 512:
                                             jc * 2048 + fc * 512 + 512],
                                         start=st, stop=(jc == 11))
                        nc.tensor.matmul(ps_g[:], src,
                                         wgt[:, jc * 2048 + fc * 512:
                                             jc * 2048 + fc * 512 + 512],
                                         start=st, stop=(jc == 11))
                    fuse = chain.tile([128, 512], F32, tag="fuse")
                    nc.scalar.activation(fuse[:], ps_f[:], AF.Tanh)
                    gate = chain.tile([128, 512], F32, tag="gate")
                    nc.scalar.activation(gate[:], ps_g[:], AF.Sigmoid)
                    cat = cat_blocks[fc]
                    cat_ap = _f32(attn[:]) if cat is None else cat[:]
                    d1 = chain.tile([128, 512], F32, tag="d1")
                    nc.vector.tensor_sub(d1[:], fuse[:], cat_ap)
                    d2 = chain.tile([128, 512], F32, tag="d2")
                    nc.vector.tensor_mul(d2[:], d1[:], gate[:])
                    d3 = chain.tile([128, 512], F32, tag="d3")
                    nc.vector.tensor_add(d3[:], d2[:], cat_ap)
                    out_t = work.tile([128, 512], F32, tag="out_t")
                    nc.vector.tensor_scalar(out_t[:], d3[:],
                                            cmc[:, ci:ci + 1], bcc[:, ci:ci + 1],
                                            op0=ALU.mult, op1=ALU.add)
                    nc.sync.dma_start(
                        io["out"][b, ci * 128:(ci + 1) * 128,
                                  fc * 512:(fc + 1) * 512], out_t[:])


_CACHE = {}


def _get_module(bpc, lc, gemm_bias):
    key = (bpc, lc, gemm_bias)
    if key in _CACHE:
        return _CACHE[key]
    nc = bacc.Bacc("TRN2", target_bir_lowering=False, debug=False,
                   num_devices=N_CORES)
    nct = lc // 128
    io = {
        "c_in": nc.dram_tensor("c_in", [bpc, lc, D], F32,
                               kind="ExternalInput").ap(),
        "q_in": nc.dram_tensor("q_in", [bpc, LQ, D], F32,
                               kind="ExternalInput").ap(),
        "w1t": nc.dram_tensor("w1t", [128, 4 * D], F32,
                              kind="ExternalInput").ap(),
        "wft": nc.dram_tensor("wft", [128, 12 * 2048], BF16,
                              kind="ExternalInput").ap(),
        "wgt": nc.dram_tensor("wgt", [128, 12 * 2048], BF16,
                              kind="ExternalInput").ap(),
        "b1c": nc.dram_tensor("b1c", [128, 4], F32, kind="ExternalInput").ap(),
        "bfr": nc.dram_tensor("bfr", [1, 2048], BF16, kind="ExternalInput").ap(),
        "bgr": nc.dram_tensor("bgr", [1, 2048], BF16, kind="ExternalInput").ap(),
        "qma": nc.dram_tensor("qma", [bpc, 128, LQ], F32,
                              kind="ExternalInput").ap(),
        "cmc": nc.dram_tensor("cmc", [bpc, 128, nct], F32,
                              kind="ExternalInput").ap(),
        "bcc": nc.dram_tensor("bcc", [bpc, 128, nct], F32,
                              kind="ExternalInput").ap(),
        "out": nc.dram_tensor("out", [bpc, lc, 4 * D], F32,
                              kind="ExternalOutput").ap(),
    }
    with tile.TileContext(nc) as tc, ExitStack() as ctx:
        _body(ctx, tc, io, bpc, lc, gemm_bias)
    nc.compile()
    _CACHE[key] = nc
    return nc


def _weff(W):
    b0, b1_, b2, b3 = (W[:, i * 512:(i + 1) * 512] for i in range(4))
    weff = np.concatenate([b0 - b2, b1_ + b2, b3], axis=1)  # [2048, 1536]
    wt = np.ascontiguousarray(weff.T)  # [1536, 2048]
    return np.ascontiguousarray(
        wt.reshape(12, 128, 2048).transpose(1, 0, 2).reshape(128, 12 * 2048)
    ).astype(BF16_NP)


def host_inputs(C, Q, Cmask, Qmask, W1, b1, Wf, bf, Wg, bg, bpc, lc, n_cores):
    nct = lc // 128
    w1t = np.ascontiguousarray(
        np.ascontiguousarray(W1.T).reshape(4, 128, D)
        .transpose(1, 0, 2).reshape(128, 4 * D), dtype=np.float32)
    wft, wgt = _weff(Wf), _weff(Wg)
    b1c = np.ascontiguousarray(b1.reshape(4, 128).T, dtype=np.float32)
    bfr = np.ascontiguousarray(bf.reshape(1, 2048), dtype=np.float32).astype(BF16_NP)
    bgr = np.ascontiguousarray(bg.reshape(1, 2048), dtype=np.float32).astype(BF16_NP)
    maps = []
    for i in range(n_cores):
        bs = slice(i * bpc, (i + 1) * bpc)
        cm = Cmask[bs].astype(np.float32)
        qm = Qmask[bs].astype(np.float32)
        cmc = np.ascontiguousarray(
            cm.reshape(bpc, nct, 128).transpose(0, 2, 1))
        bcc = (cmc - np.float32(1.0)) * np.float32(1e30)
        qma = np.ascontiguousarray(np.broadcast_to(
            ((qm - np.float32(1.0)) * np.float32(1e30))[:, None, :],
            (bpc, 128, LQ)))
        maps.append({
            "c_in": np.ascontiguousarray(C[bs], dtype=np.float32),
            "q_in": np.ascontiguousarray(Q[bs], dtype=np.float32),
            "w1t": w1t, "wft": wft, "wgt": wgt, "b1c": b1c,
            "bfr": bfr, "bgr": bgr,
            "qma": qma, "cmc": cmc, "bcc": bcc,
        })
    return maps


def kernel(C, Q, Cmask, Qmask, W1, b1, Wf, bf, Wg, bg, _trace=False):
    C = np.asarray(C, dtype=np.float32)
    Q = np.asarray(Q, dtype=np.float32)
    Cmask = np.asarray(Cmask)
    Qmask = np.asarray(Qmask)
    W1 = np.asarray(W1, dtype=np.float32)
    b1 = np.asarray(b1, dtype=np.float32)
    Wf = np.asarray(Wf, dtype=np.float32)
    bf = np.asarray(bf, dtype=np.float32)
    Wg = np.asarray(Wg, dtype=np.float32)
    bg = np.asarray(bg, dtype=np.float32)

    gemm_bias = bool(np.any(bf) or np.any(bg))
    nc = _get_module(BPC, LC_FULL, gemm_bias)
    maps = host_inputs(C, Q, Cmask, Qmask, W1, b1, Wf, bf, Wg, bg,
                       BPC, LC_FULL, N_CORES)
    res = run_bass_kernel_spmd(nc, maps, list(range(N_CORES)), trace=_trace)
    out = np.concatenate([r["out"] for r in res.results], axis=0)
    if _trace:
        return out, res
    return out


# revision 6
# speedup vs baseline: 1.0080x; 1.0080x over previous
"""BiAttention Trainium2 kernel (8 NeuronCores, data-parallel over batch).

Per core: 2 batches. Per batch b:
  C_ = lrelu(C @ W1.T + b1); Q_ = lrelu(Q @ W1.T + b1)     [fp32r matmuls]
  S  = C_ @ Q_.T, masked (Cmask rows / Qmask cols -> -1e30), softmax over Lq
  A  = S_ @ Q                                              [fp32r]
  cat = [C, A, A-C, A*C]
  fuse = tanh(cat @ Wf.T + bf); gate = sigmoid(cat @ Wg.T + bg)   [bf16]
  out = gate*fuse + (1-gate)*cat, rows with Cmask==0 -> -1e30

The 4-block cat GEMM is folded into 3 blocks on the host:
  cat @ W.T = C @ (W0-W2).T + A @ (W1+W2).T + (A*C) @ W3.T
where Wk = W[:, k*512:(k+1)*512].

Layouts (SBUF, partition dim first):
  ct_grp  [128, 4k x 512c]  fp32r  C^T for a 4-c-tile group (k = C feature chunk)
  cgrp    [128, 4d x 512c]  fp32r  C_^T for the group (d = output feature chunk)
  q_t     [128, 4d x 512q]  fp32r  Q_^T
  qb      [128, 4qc x 512d] fp32r  Q natural (q-chunk on partitions)
  wft/wgt [128, 12j x 2048f] bf16  folded weights, moving operand
"""
import sys

sys.path.insert(0, "/opt/trn_rl_repo")

import numpy as np
import ml_dtypes
from contextlib import ExitStack

import concourse.bass as bass
import concourse.tile as tile
from concourse import bacc, mybir
from concourse.bass_utils import run_bass_kernel_spmd
from concourse.masks import make_identity

F32 = mybir.dt.float32
F32R = mybir.dt.float32r
BF16 = mybir.dt.bfloat16
AF = mybir.ActivationFunctionType
ALU = mybir.AluOpType
AX = mybir.AxisListType
BF16_NP = ml_dtypes.bfloat16

N_CORES = 8
B_FULL, LC_FULL, LQ, D = 16, 2048, 512, 512
BPC = B_FULL // N_CORES


def _f32(ap):
    return ap.bitcast(F32)


def _body(ctx: ExitStack, tc, io, bpc, lc, gemm_bias, repeat=1):
    nc = tc.nc
    nct = lc // 128
    ngrp = nct // 4

    cons = ctx.enter_context(tc.tile_pool(name="cons", bufs=1))
    batch = ctx.enter_context(tc.tile_pool(name="batch", bufs=1))
    grp = ctx.enter_context(tc.tile_pool(name="grp", bufs=1))
    grp2 = ctx.enter_context(tc.tile_pool(name="grp2", bufs=2))
    chain = ctx.enter_context(tc.tile_pool(name="chain", bufs=1))
    psA = ctx.enter_context(tc.tile_pool(name="psA", bufs=2, space="PSUM"))
    psB = ctx.enter_context(tc.tile_pool(name="psB", bufs=2, space="PSUM"))
    psG = ctx.enter_context(tc.tile_pool(name="psG", bufs=4, space="PSUM"))

    # ---- constants ----
    wf_ch, wg_ch = [], []
    for jc in range(12):
        wf = cons.tile([128, 2048], BF16, tag=f"wft{jc}")
        wg = cons.tile([128, 2048], BF16, tag=f"wgt{jc}")
        nc.sync.dma_start(wf[:], io["wft"][:, jc * 2048:(jc + 1) * 2048])
        nc.sync.dma_start(wg[:], io["wgt"][:, jc * 2048:(jc + 1) * 2048])
        wf_ch.append(wf)
        wg_ch.append(wg)
    w1t = cons.tile([128, 4 * D], F32R, tag="w1t")
    with tc.tile_pool(name="init", bufs=1) as initp:
        w1t_f = initp.tile([128, 4 * D], F32, tag="w1t_f")
        nc.sync.dma_start(w1t_f[:], io["w1t"])
        nc.vector.tensor_copy(w1t[:], w1t_f[:])
    work = ctx.enter_context(tc.tile_pool(name="work", bufs=2))
    b1c = cons.tile([128, 4], F32, tag="b1c")
    nc.sync.dma_start(b1c[:], io["b1c"])
    ident = cons.tile([128, 128], F32, tag="ident")
    make_identity(nc, ident[:])
    ident_r = cons.tile([128, 128], F32R, tag="ident_r")
    nc.vector.tensor_copy(ident_r[:], ident[:])
    if gemm_bias:
        bfr = cons.tile([1, 2048], BF16, tag="bfr")
        bgr = cons.tile([1, 2048], BF16, tag="bgr")
        nc.sync.dma_start(bfr[:], io["bfr"])
        nc.sync.dma_start(bgr[:], io["bgr"])
        ones = cons.tile([1, 128], BF16, tag="ones")
        nc.vector.memset(ones[:], 1.0)

    for b in [bb for _ in range(repeat) for bb in range(bpc)]:
        # ---- per-batch masks ----
        qma = batch.tile([128, LQ], F32, tag="qma")
        nc.sync.dma_start(qma[:], io["qma"][b])
        cmc = batch.tile([128, nct], F32, tag="cmc")
        nc.sync.dma_start(cmc[:], io["cmc"][b])
        bcc = batch.tile([128, nct], F32, tag="bcc")
        nc.sync.dma_start(bcc[:], io["bcc"][b])

        # ---- Q prep: qb (natural, fp32r), q_t (Q_^T), staged per q-half ----
        qb = batch.tile([128, 4 * D], F32R, tag="qb")
        q_t = batch.tile([128, 4 * LQ], F32R, tag="q_t")
        qprep = tc.tile_pool(name=f"qprep{b}", bufs=1)
        qtp = qprep.__enter__()
        for h in range(2):
            qt = qtp.tile([128, 4 * 256], F32R, tag="qt")
            for qi in range(2):
                qc = 2 * h + qi
                q_nat = work.tile([128, D], F32, tag="q_nat")
                nc.sync.dma_start(q_nat[:],
                                  io["q_in"][b, qc * 128:(qc + 1) * 128, :])
                nc.vector.tensor_copy(qb[:, qc * D:(qc + 1) * D], q_nat[:])
                ps = psB.tile([128, 512], F32, tag="psB")
                for k in range(4):
                    nc.tensor.transpose(ps[:, k * 128:(k + 1) * 128],
                                        q_nat[:, k * 128:(k + 1) * 128], ident[:])
                dst = qt[:].rearrange("p (k q) -> p k q", k=4)[:, :,
                                                              qi * 128:(qi + 1) * 128]
                nc.vector.tensor_copy(dst, ps[:].rearrange("p (k q) -> p k q", k=4))
            for dc in range(4):
                ps = psA.tile([128, 256], F32, tag="psA")
                for k in range(4):
                    nc.tensor.matmul(
                        ps[:], w1t[:, k * D + dc * 128:k * D + dc * 128 + 128],
                        qt[:, k * 256:(k + 1) * 256],
                        start=(k == 0), stop=(k == 3))
                tmp = chain.tile([128, 256], F32, tag="tmp")
                nc.scalar.activation(tmp[:], ps[:], AF.Identity,
                                     bias=b1c[:, dc:dc + 1], scale=1.0)
                nc.vector.scalar_tensor_tensor(
                    q_t[:, dc * LQ + h * 256:dc * LQ + h * 256 + 256],
                    tmp[:], 0.01, tmp[:], op0=ALU.mult, op1=ALU.max)
        qprep.__exit__(None, None, None)

        for g in range(ngrp):
            # ---- group prologue: C^T and C_^T for 4 c-tiles ----
            ct_grp = grp2.tile([128, 4 * 512], F32R, tag="ct_grp")
            for t in range(4):
                ci = g * 4 + t
                c_nat = work.tile([128, D], F32, tag="c_nat0")
                nc.sync.dma_start(c_nat[:], io["c_in"][b, ci * 128:(ci + 1) * 128, :])
                ps = psB.tile([128, 512], F32, tag="psB")
                for k in range(4):
                    nc.tensor.transpose(ps[:, k * 128:(k + 1) * 128],
                                        c_nat[:, k * 128:(k + 1) * 128], ident[:])
                dst = ct_grp[:].rearrange("p (k c) -> p k c", k=4)[:, :,
                                                                  t * 128:(t + 1) * 128]
                nc.vector.tensor_copy(dst, ps[:].rearrange("p (k c) -> p k c", k=4))
            cgrp = grp.tile([128, 4 * 512], F32R, tag="cgrp")
            for dc in range(4):
                ps = psA.tile([128, 512], F32, tag="psA")
                for k in range(4):
                    nc.tensor.matmul(
                        ps[:], w1t[:, k * D + dc * 128:k * D + dc * 128 + 128],
                        ct_grp[:, k * 512:(k + 1) * 512],
                        start=(k == 0), stop=(k == 3))
                tmp = chain.tile([128, 512], F32, tag="tmp")
                nc.scalar.activation(tmp[:], ps[:], AF.Identity,
                                     bias=b1c[:, dc:dc + 1], scale=1.0)
                nc.vector.scalar_tensor_tensor(cgrp[:, dc * 512:(dc + 1) * 512],
                                               tmp[:], 0.01, tmp[:],
                                               op0=ALU.mult, op1=ALU.max)

            for t in range(4):
                ci = g * 4 + t
                # ---- S + masked softmax ----
                ps_s = psA.tile([128, 512], F32, tag="psA")
                for dc in range(4):
                    nc.tensor.matmul(
                        ps_s[:], cgrp[:, dc * 512 + t * 128:dc * 512 + t * 128 + 128],
                        q_t[:, dc * LQ:(dc + 1) * LQ],
                        start=(dc == 0), stop=(dc == 3))
                s1 = chain.tile([128, 512], F32, tag="s1")
                nc.vector.tensor_add(s1[:], ps_s[:], qma[:])
                s2 = chain.tile([128, 512], F32, tag="s2")
                nc.scalar.activation(s2[:], s1[:], AF.Identity,
                                     bias=bcc[:, ci:ci + 1], scale=cmc[:, ci:ci + 1])
                negm = chain.tile([128, 1], F32, tag="negm")
                nc.vector.reduce_max(negm[:], s2[:], axis=AX.X, negate=True)
                p_f = chain.tile([128, 512], F32, tag="p_f")
                ssum = chain.tile([128, 1], F32, tag="ssum")
                nc.scalar.activation(p_f[:], s2[:], AF.Exp, bias=negm[:], scale=1.0,
                                     accum_out=ssum[:])
                rec = chain.tile([128, 1], F32, tag="rec")
                nc.vector.reciprocal(rec[:], ssum[:])
                pb = chain.tile([128, 512], F32R, tag="pb")
                nc.scalar.activation(pb[:], p_f[:], AF.Copy, bias=0.0, scale=rec[:])
                # ---- P^T ----
                ps_pt = psB.tile([128, 512], F32R, tag="psB")
                for qc in range(4):
                    nc.tensor.transpose(ps_pt[:, qc * 128:(qc + 1) * 128],
                                        pb[:, qc * 128:(qc + 1) * 128], ident_r[:])
                pt = chain.tile([128, 512], F32R, tag="pt")
                nc.vector.tensor_copy(pt[:], _f32(ps_pt[:]))
                # ---- attn natural [c, d], then attn^T chunks ----
                ps_an = psB.tile([128, 512], F32, tag="psB")
                for qc in range(4):
                    nc.tensor.matmul(ps_an[:], pt[:, qc * 128:(qc + 1) * 128],
                                     qb[:, qc * D:(qc + 1) * D],
                                     start=(qc == 0), stop=(qc == 3))
                attn = work.tile([128, 512], F32R, tag="attn")
                nc.vector.tensor_copy(attn[:], ps_an[:])
                ps_at = psB.tile([128, 512], F32R, tag="psB")
                for dc in range(4):
                    nc.tensor.transpose(ps_at[:, dc * 128:(dc + 1) * 128],
                                        attn[:, dc * 128:(dc + 1) * 128], ident_r[:])
                attnt = work.tile([128, 512], BF16, tag="attnt")
                nc.vector.tensor_copy(attnt[:], _f32(ps_at[:]))
                ct_sl = _f32(ct_grp[:]).rearrange("p (k c) -> p k c", k=4)[
                    :, :, t * 128:(t + 1) * 128]
                multt = work.tile([128, 512], BF16, tag="multt")
                nc.vector.tensor_mul(multt[:].rearrange("p (k c) -> p k c", k=4),
                                     _f32(ps_at[:]).rearrange("p (k c) -> p k c", k=4),
                                     ct_sl)
                ctbf = work.tile([128, 512], BF16, tag="ctbf")
                nc.vector.tensor_copy(ctbf[:].rearrange("p (k c) -> p k c", k=4),
                                      ct_sl)
                # ---- cat natural blocks ----
                c_nat = work.tile([128, D], F32, tag="c_nat1")
                nc.sync.dma_start(c_nat[:], io["c_in"][b, ci * 128:(ci + 1) * 128, :])
                amc = work.tile([128, 512], F32, tag="amc")
                nc.vector.tensor_sub(amc[:], _f32(attn[:]), c_nat[:])
                amm = work.tile([128, 512], F32, tag="amm")
                nc.vector.tensor_mul(amm[:], _f32(attn[:]), c_nat[:])
                cat_blocks = [c_nat, None, amc, amm]
                # ---- the two big GEMMs + blend, per 512-wide f chunk ----
                for fc in range(4):
                    ps_f = psG.tile([128, 512], F32, tag="psG")
                    ps_g = psG.tile([128, 512], F32, tag="psG")
                    if gemm_bias:
                        nc.tensor.matmul(ps_f[:], ones[:, :],
                                         bfr[:, fc * 512:(fc + 1) * 512],
                                         start=True, stop=False)
                        nc.tensor.matmul(ps_g[:], ones[:, :],
                                         bgr[:, fc * 512:(fc + 1) * 512],
                                         start=True, stop=False)
                    for jc in range(12):
                        if jc < 4:
                            src = ctbf[:, (jc % 4) * 128:(jc % 4) * 128 + 128]
                        elif jc < 8:
                            src = attnt[:, (jc % 4) * 128:(jc % 4) * 128 + 128]
                        else:
                            src = multt[:, (jc % 4) * 128:(jc % 4) * 128 + 128]
                        st = (jc == 0) and not gemm_bias
                        nc.tensor.matmul(ps_f[:], src,
                                         wf_ch[jc][:, fc * 512:fc * 512 + 512],
                                         start=st, stop=(jc == 11))
                        nc.tensor.matmul(ps_g[:], src,
                                         wg_ch[jc][:, fc * 512:fc * 512 + 512],
                                         start=st, stop=(jc == 11))
                    fuse = chain.tile([128, 512], F32, tag="fuse")
                    nc.scalar.activation(fuse[:], ps_f[:], AF.Tanh)
                    gate = chain.tile([128, 512], F32, tag="gate")
                    nc.scalar.activation(gate[:], ps_g[:], AF.Sigmoid)
                    cat = cat_blocks[fc]
                    cat_ap = _f32(attn[:]) if cat is None else cat[:]
                    d1 = chain.tile([128, 512], F32, tag="d1")
                    nc.vector.tensor_sub(d1[:], fuse[:], cat_ap)
                    d2 = chain.tile([128, 512], F32, tag="d2")
                    nc.vector.tensor_mul(d2[:], d1[:], gate[:])
                    d3 = chain.tile([128, 512], F32, tag="d3")
                    nc.vector.tensor_add(d3[:], d2[:], cat_ap)
                    out_t = work.tile([128, 512], F32, tag="out_t")
                    nc.vector.tensor_scalar(out_t[:], d3[:],
                                            cmc[:, ci:ci + 1], bcc[:, ci:ci + 1],
                                            op0=ALU.mult, op1=ALU.add)
                    nc.sync.dma_start(
                        io["out"][b, ci * 128:(ci + 1) * 128,
                                  fc * 512:(fc + 1) * 512], out_t[:])


_CACHE = {}


def _get_module(bpc, lc, gemm_bias, repeat=1):
    key = (bpc, lc, gemm_bias, repeat)
    if key in _CACHE:
        return _CACHE[key]
    nc = bacc.Bacc("TRN2", target_bir_lowering=False, debug=False,
                   num_devices=N_CORES)
    nct = lc // 128
    io = {
        "c_in": nc.dram_tensor("c_in", [bpc, lc, D], F32,
                               kind="ExternalInput").ap(),
        "q_in": nc.dram_tensor("q_in", [bpc, LQ, D], F32,
                               kind="ExternalInput").ap(),
        "w1t": nc.dram_tensor("w1t", [128, 4 * D], F32,
                              kind="ExternalInput").ap(),
        "wft": nc.dram_tensor("wft", [128, 12 * 2048], BF16,
                              kind="ExternalInput").ap(),
        "wgt": nc.dram_tensor("wgt", [128, 12 * 2048], BF16,
                              kind="ExternalInput").ap(),
        "b1c": nc.dram_tensor("b1c", [128, 4], F32, kind="ExternalInput").ap(),
        "bfr": nc.dram_tensor("bfr", [1, 2048], BF16, kind="ExternalInput").ap(),
        "bgr": nc.dram_tensor("bgr", [1, 2048], BF16, kind="ExternalInput").ap(),
        "qma": nc.dram_tensor("qma", [bpc, 128, LQ], F32,
                              kind="ExternalInput").ap(),
        "cmc": nc.dram_tensor("cmc", [bpc, 128, nct], F32,
                              kind="ExternalInput").ap(),
        "bcc": nc.dram_tensor("bcc", [bpc, 128, nct], F32,
                              kind="ExternalInput").ap(),
        "out": nc.dram_tensor("out", [bpc, lc, 4 * D], F32,
                              kind="ExternalOutput").ap(),
    }
    with tile.TileContext(nc) as tc, ExitStack() as ctx:
        _body(ctx, tc, io, bpc, lc, gemm_bias, repeat)
    nc.compile()
    _CACHE[key] = nc
    return nc


def _weff(W):
    b0, b1_, b2, b3 = (W[:, i * 512:(i + 1) * 512] for i in range(4))
    weff = np.concatenate([b0 - b2, b1_ + b2, b3], axis=1)  # [2048, 1536]
    wt = np.ascontiguousarray(weff.T)  # [1536, 2048]
    return np.ascontiguousarray(
        wt.reshape(12, 128, 2048).transpose(1, 0, 2).reshape(128, 12 * 2048)
    ).astype(BF16_NP)


def host_inputs(C, Q, Cmask, Qmask, W1, b1, Wf, bf, Wg, bg, bpc, lc, n_cores):
    nct = lc // 128
    w1t = np.ascontiguousarray(
        np.ascontiguousarray(W1.T).reshape(4, 128, D)
        .transpose(1, 0, 2).reshape(128, 4 * D), dtype=np.float32)
    wft, wgt = _weff(Wf), _weff(Wg)
    b1c = np.ascontiguousarray(b1.reshape(4, 128).T, dtype=np.float32)
    bfr = np.ascontiguousarray(bf.reshape(1, 2048), dtype=np.float32).astype(BF16_NP)
    bgr = np.ascontiguousarray(bg.reshape(1, 2048), dtype=np.float32).astype(BF16_NP)
    maps = []
    for i in range(n_cores):
        bs = slice(i * bpc, (i + 1) * bpc)
        cm = Cmask[bs].astype(np.float32)
        qm = Qmask[bs].astype(np.float32)
        cmc = np.ascontiguousarray(
            cm.reshape(bpc, nct, 128).transpose(0, 2, 1))
        bcc = (cmc - np.float32(1.0)) * np.float32(1e30)
        qma = np.ascontiguousarray(np.broadcast_to(
            ((qm - np.float32(1.0)) * np.float32(1e30))[:, None, :],
            (bpc, 128, LQ)))
        maps.append({
            "c_in": np.ascontiguousarray(C[bs], dtype=np.float32),
            "q_in": np.ascontiguousarray(Q[bs], dtype=np.float32),
            "w1t": w1t, "wft": wft, "wgt": wgt, "b1c": b1c,
            "bfr": bfr, "bgr": bgr,
            "qma": qma, "cmc": cmc, "bcc": bcc,
        })
    return maps


def kernel(C, Q, Cmask, Qmask, W1, b1, Wf, bf, Wg, bg, _trace=False):
    C = np.asarray(C, dtype=np.float32)
    Q = np.asarray(Q, dtype=np.float32)
    Cmask = np.asarray(Cmask)
    Qmask = np.asarray(Qmask)
    W1 = np.asarray(W1, dtype=np.float32)
    b1 = np.asarray(b1, dtype=np.float32)
    Wf = np.asarray(Wf, dtype=np.float32)
    bf = np.asarray(bf, dtype=np.float32)
    Wg = np.asarray(Wg, dtype=np.float32)
    bg = np.asarray(bg, dtype=np.float32)

    gemm_bias = bool(np.any(bf) or np.any(bg))
    nc = _get_module(BPC, LC_FULL, gemm_bias)
    maps = host_inputs(C, Q, Cmask, Qmask, W1, b1, Wf, bf, Wg, bg,
                       BPC, LC_FULL, N_CORES)
    res = run_bass_kernel_spmd(nc, maps, list(range(N_CORES)), trace=_trace)
    out = np.concatenate([r["out"] for r in res.results], axis=0)
    if _trace:
        return out, res
    return out


# revision 15
# speedup vs baseline: 9665.3477x; 9588.3419x over previous
"""BiAttention Trainium2 kernel (8 NeuronCores, data-parallel over batch).

Per core: 2 batches. Per batch b:
  C_ = lrelu(C @ W1.T + b1); Q_ = lrelu(Q @ W1.T + b1)     [fp32r matmuls]
  S  = C_ @ Q_.T, masked (Cmask rows / Qmask cols -> -1e30), softmax over Lq
  A  = S_ @ Q                                              [fp32r]
  cat = [C, A, A-C, A*C]
  fuse = tanh(cat @ Wf.T + bf); gate = sigmoid(cat @ Wg.T + bg)   [bf16]
  out = gate*fuse + (1-gate)*cat, rows with Cmask==0 -> -1e30

The 4-block cat GEMM is folded into 3 blocks on the host:
  cat @ W.T = C @ (W0-W2).T + A @ (W1+W2).T + (A*C) @ W3.T
where Wk = W[:, k*512:(k+1)*512].

sigmoid(x) = 0.5*tanh(x/2)+0.5 so every ACT function used (Identity/Copy/
Exp/Tanh) lives in one table set ("exp_and_others") -- no table reloads.

Emission is software-pipelined: each c-tile's attention chain (4 parts:
S+softmax, P^T, attn, attn^T+cat blocks) and each group's C^T/C_^T
prologue are emitted in slots between earlier tiles' GEMM psum groups,
so the PE queue never drains waiting on the scalar/vector softmax chain.
"""
import sys

sys.path.insert(0, "/opt/trn_rl_repo")

from collections import deque
from contextlib import ExitStack

import numpy as np
import ml_dtypes

import concourse.bass as bass
import concourse.tile as tile
from concourse import bacc, mybir
from concourse.bass_utils import run_bass_kernel_spmd
from concourse.masks import make_identity

F32 = mybir.dt.float32
F32R = mybir.dt.float32r
BF16 = mybir.dt.bfloat16
AF = mybir.ActivationFunctionType
ALU = mybir.AluOpType
AX = mybir.AxisListType
BF16_NP = ml_dtypes.bfloat16

N_CORES = 8
B_FULL, LC_FULL, LQ, D = 16, 2048, 512, 512
BPC = B_FULL // N_CORES


def _f32(ap):
    return ap.bitcast(F32)


def _body(ctx: ExitStack, tc, io, bpc, lc, gemm_bias, repeat=1):
    nc = tc.nc
    nct = lc // 128
    ngrp = nct // 4

    cons = ctx.enter_context(tc.tile_pool(name="cons", bufs=1))
    batch = ctx.enter_context(tc.tile_pool(name="batch", bufs=1))
    grp = ctx.enter_context(tc.tile_pool(name="grp", bufs=1))
    grp2 = ctx.enter_context(tc.tile_pool(name="grp2", bufs=2))
    cn_pool = ctx.enter_context(
        tc.tile_pool(name="cn", bufs=3 if gemm_bias else 4))
    chain = ctx.enter_context(tc.tile_pool(name="chain", bufs=1))
    psA = ctx.enter_context(tc.tile_pool(name="psA", bufs=2, space="PSUM"))
    psB = ctx.enter_context(tc.tile_pool(name="psB", bufs=2, space="PSUM"))
    psG = ctx.enter_context(tc.tile_pool(name="psG", bufs=4, space="PSUM"))

    # ---- weights: DMAs drip-fed at critical emission points (b==0 only) ----
    wf_ch, wg_ch = [], []
    pending_w = []
    for jc in range(12):
        wf = cons.tile([128, 2048], BF16, tag=f"wft{jc}")
        wg = cons.tile([128, 2048], BF16, tag=f"wgt{jc}")
        pending_w.append((wf, io["wft"][:, jc * 2048:(jc + 1) * 2048]))
        pending_w.append((wg, io["wgt"][:, jc * 2048:(jc + 1) * 2048]))
        wf_ch.append(wf)
        wg_ch.append(wg)

    def drip_w(npairs):
        for _ in range(2 * npairs):
            if pending_w:
                t, src = pending_w.pop(0)
                nc.sync.dma_start(t[:], src)

    w1t = cons.tile([128, 4 * D], F32R, tag="w1t")
    b1c = cons.tile([128, 4], F32, tag="b1c")
    nc.sync.dma_start(b1c[:], io["b1c"])
    ident = cons.tile([128, 128], F32, tag="ident")
    make_identity(nc, ident[:])
    ident_r = cons.tile([128, 128], F32R, tag="ident_r")
    nc.vector.tensor_copy(ident_r[:], ident[:])
    halfc = cons.tile([128, 1], F32, tag="halfc")
    nc.vector.memset(halfc[:], 0.5)
    if gemm_bias:
        bfg = cons.tile([64, 2048], BF16, tag="bfg")
        nc.sync.dma_start(bfg[0:1, :], io["bfr"])
        nc.sync.dma_start(bfg[32:33, :], io["bgr"])
        ones = cons.tile([64, 128], BF16, tag="ones")
        nc.vector.memset(ones[:], 1.0)
    # stage w1t fp32 -> fp32r via a closing pool (frees 8KB for `work`)
    with tc.tile_pool(name="init", bufs=1) as initp:
        w1t_f = initp.tile([128, 4 * D], F32, tag="w1t_f")
        for k in range(4):
            nc.sync.dma_start(w1t_f[:, k * 512:(k + 1) * 512],
                              io["w1t"][:, k * 512:(k + 1) * 512])
        nc.vector.tensor_copy(w1t[:], w1t_f[:])
    work = ctx.enter_context(tc.tile_pool(name="work", bufs=2))

    def qprep_emit(b, first):
        """Masks + Q natural (fp32r) + Q_^T staging. Serial per batch."""
        qma = batch.tile([128, LQ], BF16, tag="qma")
        cmc = batch.tile([128, nct], F32, tag="cmc")
        bcc = batch.tile([128, nct], F32, tag="bcc")
        qb = batch.tile([128, 4 * D], F32R, tag="qb")
        q_t = batch.tile([128, 4 * LQ], F32R, tag="q_t")
        nc.sync.dma_start(qma[:], io["qma"][b])
        nc.sync.dma_start(cmc[:], io["cmc"][b])
        nc.sync.dma_start(bcc[:], io["bcc"][b])
        qprep = tc.tile_pool(name=f"qprep{b}_{id(qb)}", bufs=1)
        qtp = qprep.__enter__()
        for h in range(2):
            qt = qtp.tile([128, 4 * 256], F32R, tag="qt")
            for qi in range(2):
                qc_i = 2 * h + qi
                q_nat = work.tile([128, D], F32, tag="q_nat")
                nc.sync.dma_start(q_nat[:],
                                  io["q_in"][b, qc_i * 128:(qc_i + 1) * 128, :])
                if first:
                    drip_w(1)
                nc.vector.tensor_copy(qb[:, qc_i * D:(qc_i + 1) * D], q_nat[:])
                ps = psB.tile([128, 512], F32, tag="psB")
                for k in range(4):
                    nc.tensor.transpose(ps[:, k * 128:(k + 1) * 128],
                                        q_nat[:, k * 128:(k + 1) * 128],
                                        ident[:])
                dst = qt[:].rearrange("p (k q) -> p k q", k=4)[
                    :, :, qi * 128:(qi + 1) * 128]
                nc.vector.tensor_copy(dst,
                                      ps[:].rearrange("p (k q) -> p k q", k=4))
            for dc in range(4):
                ps = psA.tile([128, 256], F32, tag="psA")
                for k in range(4):
                    nc.tensor.matmul(
                        ps[:], w1t[:, k * D + dc * 128:k * D + dc * 128 + 128],
                        qt[:, k * 256:(k + 1) * 256],
                        start=(k == 0), stop=(k == 3))
                tmp = chain.tile([128, 256], F32, tag="tmp")
                nc.scalar.activation(tmp[:], ps[:], AF.Identity,
                                     bias=b1c[:, dc:dc + 1], scale=1.0)
                nc.vector.scalar_tensor_tensor(
                    q_t[:, dc * LQ + h * 256:dc * LQ + h * 256 + 256],
                    tmp[:], 0.01, tmp[:], op0=ALU.mult, op1=ALU.max)
        qprep.__exit__(None, None, None)
        return {"qma": qma, "cmc": cmc, "bcc": bcc, "qb": qb, "q_t": q_t}

    def emit_cnat_dmas(b, g, drip=0):
        tiles = []
        for t in range(4):
            ci = g * 4 + t
            cp = cn_pool.tile([128, D], F32, tag="c_nat0")
            nc.sync.dma_start(cp[:], io["c_in"][b, ci * 128:(ci + 1) * 128, :])
            if drip:
                drip_w(drip)
            tiles.append(cp)
        return tiles

    def make_prologue_parts(b, g, cn_tiles):
        """4 closures: [transpose t01, transpose t23, C_ dc01, C_ dc23]."""
        st = {}

        def tr(half):
            def f():
                if "ct" not in st:
                    st["ct"] = grp2.tile([128, 4 * 512], F32R, tag="ct_grp", name="ct_grp")
                for t in (2 * half, 2 * half + 1):
                    c_nat = cn_tiles[t]
                    ps = psB.tile([128, 512], F32, tag="psB")
                    for k in range(4):
                        nc.tensor.transpose(ps[:, k * 128:(k + 1) * 128],
                                            c_nat[:, k * 128:(k + 1) * 128],
                                            ident[:])
                    dst = st["ct"][:].rearrange("p (k c) -> p k c", k=4)[
                        :, :, t * 128:(t + 1) * 128]
                    nc.vector.tensor_copy(
                        dst, ps[:].rearrange("p (k c) -> p k c", k=4))
            return f

        def cmm(half):
            def f():
                if "cg" not in st:
                    st["cg"] = grp.tile([128, 4 * 512], F32R, tag="cgrp", name="cgrp")
                for dc in (2 * half, 2 * half + 1):
                    ps = psA.tile([128, 512], F32, tag="psA")
                    for k in range(4):
                        nc.tensor.matmul(
                            ps[:],
                            w1t[:, k * D + dc * 128:k * D + dc * 128 + 128],
                            st["ct"][:, k * 512:(k + 1) * 512],
                            start=(k == 0), stop=(k == 3))
                    tmp = chain.tile([128, 512], F32, tag="tmp")
                    nc.scalar.activation(tmp[:], ps[:], AF.Identity,
                                         bias=b1c[:, dc:dc + 1], scale=1.0)
                    nc.vector.scalar_tensor_tensor(
                        st["cg"][:, dc * 512:(dc + 1) * 512],
                        tmp[:], 0.01, tmp[:], op0=ALU.mult, op1=ALU.max)
            return f

        return [tr(0), tr(1), cmm(0), cmm(1)], st

    def make_chain_parts(b, g, t, pro_st, qc, st):
        ci = g * 4 + t

        def p0():  # S + masked softmax -> pb (fp32r, scaled by 1/sum)
            ps_s = psA.tile([128, 512], F32, tag="psA")
            for dc in range(4):
                nc.tensor.matmul(
                    ps_s[:],
                    pro_st["cg"][:, dc * 512 + t * 128:dc * 512 + t * 128 + 128],
                    qc["q_t"][:, dc * LQ:(dc + 1) * LQ],
                    start=(dc == 0), stop=(dc == 3))
            s1 = chain.tile([128, 512], F32, tag="s1")
            nc.vector.tensor_add(s1[:], ps_s[:], qc["qma"][:])
            s2 = chain.tile([128, 512], F32, tag="s2")
            nc.scalar.activation(s2[:], s1[:], AF.Identity,
                                 bias=qc["bcc"][:, ci:ci + 1],
                                 scale=qc["cmc"][:, ci:ci + 1])
            negm = chain.tile([128, 1], F32, tag="negm")
            nc.vector.reduce_max(negm[:], s2[:], axis=AX.X, negate=True)
            p_f = chain.tile([128, 512], F32, tag="s1")
            ssum = chain.tile([128, 1], F32, tag="ssum")
            nc.scalar.activation(p_f[:], s2[:], AF.Exp, bias=negm[:],
                                 scale=1.0, accum_out=ssum[:])
            rec = chain.tile([128, 1], F32, tag="rec")
            nc.vector.reciprocal(rec[:], ssum[:])
            pb = chain.tile([128, 512], F32R, tag="pb")
            nc.scalar.activation(pb[:], p_f[:], AF.Copy, bias=0.0,
                                 scale=rec[:])
            st["pb"] = pb

        def p1():  # P^T
            ps_pt = psB.tile([128, 512], F32R, tag="psB")
            for qq in range(4):
                nc.tensor.transpose(ps_pt[:, qq * 128:(qq + 1) * 128],
                                    st["pb"][:, qq * 128:(qq + 1) * 128],
                                    ident_r[:])
            pt = chain.tile([128, 512], F32R, tag="s2")
            nc.vector.tensor_copy(pt[:], _f32(ps_pt[:]))
            st["pt"] = pt

        def p2():  # attn natural + blend C reload
            ps_an = psB.tile([128, 512], F32, tag="psB")
            for qq in range(4):
                nc.tensor.matmul(ps_an[:],
                                 st["pt"][:, qq * 128:(qq + 1) * 128],
                                 qc["qb"][:, qq * D:(qq + 1) * D],
                                 start=(qq == 0), stop=(qq == 3))
            attn = work.tile([128, 512], F32R, tag="attn")
            nc.vector.tensor_copy(attn[:], ps_an[:])
            st["attn"] = attn
            c_nat = work.tile([128, D], F32, tag="c_nat1")
            nc.sync.dma_start(c_nat[:],
                              io["c_in"][b, ci * 128:(ci + 1) * 128, :])
            st["c_nat"] = c_nat

        def p3():  # attn^T + T-layout cat blocks + natural cat blocks
            ps_at = psB.tile([128, 512], F32R, tag="psB")
            for dc in range(4):
                nc.tensor.transpose(ps_at[:, dc * 128:(dc + 1) * 128],
                                    st["attn"][:, dc * 128:(dc + 1) * 128],
                                    ident_r[:])
            attnt = work.tile([128, 512], BF16, tag="attnt")
            nc.vector.tensor_copy(attnt[:], _f32(ps_at[:]))
            ct_sl = _f32(pro_st["ct"][:]).rearrange("p (k c) -> p k c", k=4)[
                :, :, t * 128:(t + 1) * 128]
            multt = work.tile([128, 512], BF16, tag="multt")
            nc.vector.tensor_mul(
                multt[:].rearrange("p (k c) -> p k c", k=4),
                _f32(ps_at[:]).rearrange("p (k c) -> p k c", k=4), ct_sl)
            ctbf = work.tile([128, 512], BF16, tag="ctbf")
            nc.vector.tensor_copy(
                ctbf[:].rearrange("p (k c) -> p k c", k=4), ct_sl)
            amc = work.tile([128, 512], F32, tag="amc")
            nc.vector.tensor_sub(amc[:], _f32(st["attn"][:]), st["c_nat"][:])
            amm = work.tile([128, 512], F32, tag="amm")
            nc.vector.tensor_mul(amm[:], _f32(st["attn"][:]), st["c_nat"][:])
            st.update(attnt=attnt, multt=multt, ctbf=ctbf, amc=amc, amm=amm)

        return [p0, p1, p2, p3]

    def gemm_stage(b, g, t, ch_st, qc, pend):
        ci = g * 4 + t
        key = (b, g, t)
        # everything this tile depends on must be emitted before its MMs
        while any(k == key for k, _ in pend):
            _, part = pend.popleft()
            part()
        emitted = 0
        for fc in range(4):
            while pend and emitted < fc + 2:
                _, part = pend.popleft()
                part()
                emitted += 1
            ps_f = psG.tile([128, 512], F32, tag="psG")
            ps_g = psG.tile([128, 512], F32, tag="psG")
            if gemm_bias:
                nc.tensor.matmul(ps_f[:], ones[0:1, :],
                                 bfg[0:1, fc * 512:(fc + 1) * 512],
                                 start=True, stop=False)
                nc.tensor.matmul(ps_g[:], ones[32:33, :],
                                 bfg[32:33, fc * 512:(fc + 1) * 512],
                                 start=True, stop=False)
            for jc in range(12):
                if jc < 4:
                    src = ch_st["ctbf"][:, (jc % 4) * 128:(jc % 4) * 128 + 128]
                elif jc < 8:
                    src = ch_st["attnt"][:, (jc % 4) * 128:(jc % 4) * 128 + 128]
                else:
                    src = ch_st["multt"][:, (jc % 4) * 128:(jc % 4) * 128 + 128]
                stt = (jc == 0) and not gemm_bias
                nc.tensor.matmul(ps_f[:], src,
                                 wf_ch[jc][:, fc * 512:fc * 512 + 512],
                                 start=stt, stop=(jc == 11))
                nc.tensor.matmul(ps_g[:], src,
                                 wg_ch[jc][:, fc * 512:fc * 512 + 512],
                                 start=stt, stop=(jc == 11))
            fuse = chain.tile([128, 512], F32, tag="fuse")
            nc.scalar.activation(fuse[:], ps_f[:], AF.Tanh)
            gth = chain.tile([128, 512], F32, tag="gth")
            nc.scalar.activation(gth[:], ps_g[:], AF.Tanh, scale=0.5)
            gate = chain.tile([128, 512], F32, tag="gate")
            nc.scalar.activation(gate[:], gth[:], AF.Identity,
                                 bias=halfc[:], scale=0.5)
            cat_ap = [ch_st["c_nat"][:], _f32(ch_st["attn"][:]),
                      ch_st["amc"][:], ch_st["amm"][:]][fc]
            d1 = chain.tile([128, 512], F32, tag="d1")
            nc.vector.tensor_sub(d1[:], fuse[:], cat_ap)
            d2 = chain.tile([128, 512], F32, tag="fuse")
            nc.vector.tensor_mul(d2[:], d1[:], gate[:])
            d3 = chain.tile([128, 512], F32, tag="gth")
            nc.vector.tensor_add(d3[:], d2[:], cat_ap)
            out_t = work.tile([128, 512], F32, tag="out_t")
            nc.vector.tensor_scalar(out_t[:], d3[:],
                                    qc["cmc"][:, ci:ci + 1],
                                    qc["bcc"][:, ci:ci + 1],
                                    op0=ALU.mult, op1=ALU.add)
            nc.sync.dma_start(
                io["out"][b, ci * 128:(ci + 1) * 128,
                          fc * 512:(fc + 1) * 512], out_t[:])

    seq = [bb for _ in range(repeat) for bb in range(bpc)]
    for bi, b in enumerate(seq):
        qc = qprep_emit(b, first=(bi == 0))
        cn0 = emit_cnat_dmas(b, 0, drip=2 if bi == 0 else 0)
        pro_parts, pro_st0 = make_prologue_parts(b, 0, cn0)
        for p in pro_parts:
            p()
        ch_st0 = {}
        for p in make_chain_parts(b, 0, 0, pro_st0, qc, ch_st0):
            p()

        pend = deque()
        pro_states = {0: pro_st0}
        cn_states = {}
        ch_states = {(0, 0): ch_st0}

        def queue_chain(gg, tt):
            stx = {}
            ch_states[(gg, tt)] = stx
            holder = {}

            def fmk(i):
                def f():
                    if "parts" not in holder:
                        holder["parts"] = make_chain_parts(
                            b, gg, tt, pro_states[gg], qc, stx)
                    holder["parts"][i]()
                return f
            for i in range(4):
                pend.append(((b, gg, tt), fmk(i)))

        def queue_prologue(gg):
            holder = {}

            def fmk(i):
                def f():
                    if "parts" not in holder:
                        holder["parts"], pst = make_prologue_parts(
                            b, gg, cn_states[gg])
                        pro_states[gg] = pst
                    holder["parts"][i]()
                return f
            for i in range(4):
                pend.append(((b, gg, 98), fmk(i)))

        for g in range(ngrp):
            for t in range(4):
                if t == 0 and (g, 1) not in ch_states:
                    queue_chain(g, 1)
                elif t == 1:
                    if (g, 2) not in ch_states:
                        queue_chain(g, 2)
                    if g + 1 < ngrp:
                        def _cn(gg=g + 1):
                            def f():
                                cn_states[gg] = emit_cnat_dmas(b, gg)
                            return f
                        pend.append(((b, g, 99), _cn()))
                elif t == 2:
                    if (g, 3) not in ch_states:
                        queue_chain(g, 3)
                    if g + 1 < ngrp:
                        queue_prologue(g + 1)
                elif t == 3 and g + 1 < ngrp:
                    queue_chain(g + 1, 0)

                gemm_stage(b, g, t, ch_states[(g, t)], qc, pend)
                ch_states.pop((g, t), None)
        while pend:
            pend.popleft()[1]()


_CACHE = {}


def _get_module(bpc, lc, gemm_bias, repeat=1):
    key = (bpc, lc, gemm_bias, repeat)
    if key in _CACHE:
        return _CACHE[key]
    nc = bacc.Bacc("TRN2", target_bir_lowering=False, debug=False,
                   num_devices=N_CORES)
    nct = lc // 128
    io = {
        "c_in": nc.dram_tensor("c_in", [bpc, lc, D], F32,
                               kind="ExternalInput").ap(),
        "q_in": nc.dram_tensor("q_in", [bpc, LQ, D], F32,
                               kind="ExternalInput").ap(),
        "w1t": nc.dram_tensor("w1t", [128, 4 * D], F32,
                              kind="ExternalInput").ap(),
        "wft": nc.dram_tensor("wft", [128, 12 * 2048], BF16,
                              kind="ExternalInput").ap(),
        "wgt": nc.dram_tensor("wgt", [128, 12 * 2048], BF16,
                              kind="ExternalInput").ap(),
        "b1c": nc.dram_tensor("b1c", [128, 4], F32, kind="ExternalInput").ap(),
        "bfr": nc.dram_tensor("bfr", [1, 2048], BF16,
                              kind="ExternalInput").ap(),
        "bgr": nc.dram_tensor("bgr", [1, 2048], BF16,
                              kind="ExternalInput").ap(),
        "qma": nc.dram_tensor("qma", [bpc, 128, LQ], BF16,
                              kind="ExternalInput").ap(),
        "cmc": nc.dram_tensor("cmc", [bpc, 128, nct], F32,
                              kind="ExternalInput").ap(),
        "bcc": nc.dram_tensor("bcc", [bpc, 128, nct], F32,
                              kind="ExternalInput").ap(),
        "out": nc.dram_tensor("out", [bpc, lc, 4 * D], F32,
                              kind="ExternalOutput").ap(),
    }
    with tile.TileContext(nc) as tc, ExitStack() as ctx:
        _body(ctx, tc, io, bpc, lc, gemm_bias, repeat)
    nc.compile()
    _CACHE[key] = nc
    return nc


def _weff(W):
    b0, b1_, b2, b3 = (W[:, i * 512:(i + 1) * 512] for i in range(4))
    weff = np.concatenate([b0 - b2, b1_ + b2, b3], axis=1)  # [2048, 1536]
    wt = np.ascontiguousarray(weff.T)  # [1536, 2048]
    return np.ascontiguousarray(
        wt.reshape(12, 128, 2048).transpose(1, 0, 2).reshape(128, 12 * 2048)
    ).astype(BF16_NP)


def host_inputs(C, Q, Cmask, Qmask, W1, b1, Wf, bf, Wg, bg, bpc, lc, n_cores):
    nct = lc // 128
    w1t = np.ascontiguousarray(
        np.ascontiguousarray(W1.T).reshape(4, 128, D)
        .transpose(1, 0, 2).reshape(128, 4 * D), dtype=np.float32)
    wft, wgt = _weff(Wf), _weff(Wg)
    b1c = np.ascontiguousarray(b1.reshape(4, 128).T, dtype=np.float32)
    bfr = np.ascontiguousarray(bf.reshape(1, 2048),
                               dtype=np.float32).astype(BF16_NP)
    bgr = np.ascontiguousarray(bg.reshape(1, 2048),
                               dtype=np.float32).astype(BF16_NP)
    maps = []
    for i in range(n_cores):
        bs = slice(i * bpc, (i + 1) * bpc)
        cm = Cmask[bs].astype(np.float32)
        qm = Qmask[bs].astype(np.float32)
        cmc = np.ascontiguousarray(
            cm.reshape(bpc, nct, 128).transpose(0, 2, 1))
        bcc = (cmc - np.float32(1.0)) * np.float32(1e30)
        qma = np.ascontiguousarray(np.broadcast_to(
            ((qm - np.float32(1.0)) * np.float32(1e30))[:, None, :],
            (bpc, 128, LQ))).astype(BF16_NP)
        maps.append({
            "c_in": np.ascontiguousarray(C[bs], dtype=np.float32),
            "q_in": np.ascontiguousarray(Q[bs], dtype=np.float32),
            "w1t": w1t, "wft": wft, "wgt": wgt, "b1c": b1c,
            "bfr": bfr, "bgr": bgr,
            "qma": qma, "cmc": cmc, "bcc": bcc,
        })
    return maps


def kernel(C, Q, Cmask, Qmask, W1, b1, Wf, bf, Wg, bg, _trace=False):
    C = np.asarray(C, dtype=np.float32)
    Q = np.asarray(Q, dtype=np.float32)
    Cmask = np.asarray(Cmask)
    Qmask = np.asarray(Qmask)
    W1 = np.asarray(W1, dtype=np.float32)
    b1 = np.asarray(b1, dtype=np.float32)
    Wf = np.asarray(Wf, dtype=np.float32)
    bf = np.asarray(bf, dtype=np.float32)
    Wg = np.asarray(Wg, dtype=np.float32)
    bg = np.asarray(bg, dtype=np.float32)

    gemm_bias = bool(np.any(bf) or np.any(bg))
    nc = _get_module(BPC, LC_FULL, gemm_bias)
    maps = host_inputs(C, Q, Cmask, Qmask, W1, b1, Wf, bf, Wg, bg,
                       BPC, LC_FULL, N_CORES)
    res = run_bass_kernel_spmd(nc, maps, list(range(N_CORES)), trace=_trace)
    out = np.concatenate([r["out"] for r in res.results], axis=0)
    if _trace:
        return out, res
    return out
